# revision 9
# baseline (speedup 1.0000x reference)
"""Cross-channel attention kernel for Trainium2 (8 NeuronCores).

Problem (hardcoded shapes): B=2, C=64 per color -> NF=192 channels,
H=W=96 -> N=9216 spatial positions, RD=24 query/key dim.

    rgb  = concat(r,g,b)            # [B, 192, 9216]
    q    = Wq @ rgb + bq            # [B, 24, 9216]
    k    = Wk @ rgb + bk            # [B, 24, 9216]
    v    = Wv @ rgb + bv            # [B, 192, 9216]
    attn = softmax_j(q^T k)         # [B, 9216, 9216] row-softmax over keys
    out  = rgb + v @ attn^T         # residual added on host in fp32

Sharding: data-parallel over B (2) x sequence-parallel over query rows
(4 shards of 2304) = 8 cores.  Each core computes k and v redundantly
(they're tiny) and produces out[j, c] for its 2304 query rows.

Device-side layout ("keys on partitions"):
  scoresT[n, j] = sum_r k[r, n] q[r, j]     (K=24 matmul)
  e = schraudolph_exp(scoresT)              (VectorE: bf16 bits via int16
                                             affine + bitcast; max rel err
                                             ~3% per element, cancels in the
                                             softmax ratio; attention output
                                             is ~0.3% of the residual)
  acc[j, c] += e[n, j]^T vT[n, c_aug]       (matmul over key chunks of 128)
where vT carries an all-ones column so acc[:, 192] accumulates the softmax
denominator for free; out = acc[:, :192] / acc[:, 192].  No max-subtraction:
logits are O(1) by construction (weights scaled 0.02), exp cannot overflow.

Performance structure vs the naive version:
  * The scores matmul has K=24 -- a full 128x128 matmul wastes 4/5 of the
    PE array.  The PE is addressed in 32-row tiles (tile_position=(32i,0)):
    keys are laid out in 4 quarter-blocks of 32 partitions (k4), q is
    replicated in all four 32-partition blocks (q4), and 4 score matmuls
    for 4 different key chunks run CONCURRENTLY on the four row-groups of
    the array, writing 4 different PSUM banks (~3x measured on this idiom).
  * exp runs entirely on VectorE (tensor_scalar mult+add to int16, bitcast
    to bf16).  ScalarE only does projection copies and the output scaling,
    so neither activation engine is ever the bottleneck.
  * Projection weights are replicated/masked host-side into 32-column
    blocks so every PSUM->SBUF copy is full 128-partition width, and the
    hi-slab of rgb carries an all-ones row 64 (host-side) for the biases:
    no device memsets, no 32-partition copies, PE runs back-to-back.
  * PSUM: scores 4 banks (single-buffered, sufficient because VectorE exp
    of a group is faster than the PE work per group) + accumulators 4
    banks (one 193-wide region per bank) = exactly 8 banks.
"""

import numpy as np
import ml_dtypes

BF = ml_dtypes.bfloat16

# Shapes (hardcoded per problem spec)
B = 2
C = 64
HH = 96
WW = 96
N = HH * WW            # 9216 keys
NF = 3 * C             # 192 channels
RD = 24                # q/k dim
NCORES = 8
SHARDS_PER_BATCH = 4
SHARD = N // SHARDS_PER_BATCH   # 2304 query rows per core

JTILES = [512, 512, 512, 512, 256]   # query-tile widths (sum = SHARD)
PCH = 128              # key chunk (partition dim)
NCH = N // PCH         # 72 key chunks
QUART = NCH // 4       # 18 chunks per key quarter
KHI = 65               # hi K-slab: channels 128..191 + ones row
QSH = N // 4           # 2304 keys per quarter

_last_results = None   # BassKernelResults of the most recent run (for test.py)


def _build_program():
    import concourse.tile as tile
    from concourse import bacc, mybir

    f32 = mybir.dt.float32
    bf16 = mybir.dt.bfloat16
    i16 = mybir.dt.int16
    # Schraudolph fast-exp in bf16 bit space:
    #   exp(x) ~= bitcast_bf16(int16(A*x + B)),  A = 2^7/ln2, B = 127*2^7 - c
    # (c tuned to minimize relative error; +-3% worst case, cancels in the
    # softmax ratio since the denominator uses the same approximated values)
    EXPA = float(128.0 / np.log(2.0))
    EXPB = float(127 * 128) - 5.59

    nc = bacc.Bacc()

    # rgb hi slabs carry an all-ones row 64 (built host-side) for the biases
    d_rgb_lo = nc.dram_tensor("rgb_lo", [128, N], bf16, kind="ExternalInput")
    d_rgb_hi = nc.dram_tensor("rgb_hi", [KHI, N], bf16, kind="ExternalInput")
    d_qrgb_lo = nc.dram_tensor("qrgb_lo", [128, SHARD], bf16, kind="ExternalInput")
    d_qrgb_hi = nc.dram_tensor("qrgb_hi", [KHI, SHARD], bf16, kind="ExternalInput")
    # q/k weight slabs: 4 replicated (q) or masked (k) 32-column blocks;
    # hi slabs carry the bias in row 64 (multiplied by the rgb ones row)
    d_wq0 = nc.dram_tensor("wq0", [128, 128], bf16, kind="ExternalInput")
    d_wq1 = nc.dram_tensor("wq1", [KHI, 128], bf16, kind="ExternalInput")
    d_wk0 = nc.dram_tensor("wk0", [128, 4, 128], bf16, kind="ExternalInput")
    d_wk1 = nc.dram_tensor("wk1", [KHI, 4, 128], bf16, kind="ExternalInput")
    d_wv0 = nc.dram_tensor("wv0", [128, NF + 1], bf16, kind="ExternalInput")
    d_wv1 = nc.dram_tensor("wv1", [KHI, NF + 1], bf16, kind="ExternalInput")
    d_out = nc.dram_tensor("out", [SHARD, NF], f32, kind="ExternalOutput")

    with tile.TileContext(nc) as tc:
        with (
            tc.tile_pool(name="const", bufs=1) as const,
            tc.tile_pool(name="work", bufs=3) as work,
            tc.tile_pool(name="pp", bufs=1, space="PSUM") as pp,
        ):
            # ---- SBUF tiles ----
            s_rgb_lo = const.tile([128, N], bf16)
            s_rgb_hi = const.tile([KHI, N], bf16)
            s_qrgb_lo = const.tile([128, SHARD], bf16)
            s_qrgb_hi = const.tile([KHI, SHARD], bf16)
            s_wq0 = const.tile([128, 128], bf16)
            s_wq1 = const.tile([KHI, 128], bf16)
            s_wk0 = const.tile([128, 4, 128], bf16)
            s_wk1 = const.tile([KHI, 4, 128], bf16)
            s_wv0 = const.tile([128, NF + 1], bf16)
            s_wv1 = const.tile([KHI, NF + 1], bf16)
            # k4: partition block i = key quarter i (rows: 24 k dims + 8 zero)
            # q4: q replicated in all 4 partition blocks
            s_k4 = const.tile([128, QSH], bf16)
            s_q4 = const.tile([128, SHARD], bf16)
            s_vT = const.tile([128, NCH, NF + 1], bf16)

            # PSUM: tag "s" = 4 banks (scores / proj targets), tag "acc" =
            # 4 banks (attention accumulators, one 193-wide region per bank;
            # also proj targets).  Both tiles live for the whole kernel.
            st = pp.tile([128, 4, 512], f32, tag="s", bufs=1, name="st")
            pacc = pp.tile([128, 4, 512], f32, tag="acc", bufs=1, name="pacc")

            # ---- input DMA ----
            nc.sync.dma_start(out=s_wq0[:], in_=d_wq0[:])
            nc.sync.dma_start(out=s_wq1[:], in_=d_wq1[:])
            nc.sync.dma_start(out=s_wk0[:], in_=d_wk0[:])
            nc.sync.dma_start(out=s_wk1[:], in_=d_wk1[:])
            nc.sync.dma_start(out=s_wv0[:], in_=d_wv0[:])
            nc.sync.dma_start(out=s_wv1[:], in_=d_wv1[:])
            for i in range(2):
                sl = slice(i * (SHARD // 2), (i + 1) * (SHARD // 2))
                nc.sync.dma_start(out=s_qrgb_lo[:, sl], in_=d_qrgb_lo[:, sl])
                nc.sync.dma_start(out=s_qrgb_hi[:, sl], in_=d_qrgb_hi[:, sl])
            for i in range(4):
                sl = slice(i * QSH, (i + 1) * QSH)
                nc.sync.dma_start(out=s_rgb_lo[:, sl], in_=d_rgb_lo[:, sl])
                nc.sync.dma_start(out=s_rgb_hi[:, sl], in_=d_rgb_hi[:, sl])

            # PE warmup: the HAM clock gate keeps the PE at 1.2 GHz until it
            # sees a ~3.4us busy window.  Burn zero matmuls under the input
            # DMA head so the projections run at 2.4 GHz.
            wz = const.tile([128, 512], bf16)
            nc.vector.memset(wz, 0.0)
            for w in range(10):
                nc.tensor.matmul(st[:, w % 4, :], lhsT=wz[:, :128], rhs=wz,
                                 start=True, stop=True)

            # rotate PSUM->SBUF copies between Vector and Scalar
            copy_engines = [
                lambda out, in_: nc.vector.tensor_copy(out=out, in_=in_),
                lambda out, in_: nc.scalar.copy(out=out, in_=in_),
            ]
            cctr = [0]

            def pcopy(out, in_):
                copy_engines[cctr[0] % 2](out=out, in_=in_)
                cctr[0] += 1

            # ---- projections ----
            # q: replicated weights -> q in all 4 partition blocks; one
            # full-width copy per tile.
            for m, (q0, qw) in enumerate(zip(range(0, SHARD, 512), JTILES)):
                sl = slice(q0, q0 + qw)
                pq = st[:, m % 4, :qw]
                nc.tensor.matmul(pq, lhsT=s_wq0, rhs=s_qrgb_lo[:, sl],
                                 start=True, stop=False)
                nc.tensor.matmul(pq, lhsT=s_wq1, rhs=s_qrgb_hi[:, sl],
                                 start=False, stop=True)
                pcopy(out=s_q4[:, sl], in_=pq)

            # v: vT[key, c] per key chunk + ones column.  Rotate through the
            # 4 acc banks (psum targets; proj phase only).  Quarters 0-2
            # first (they overlap the rgb DMA), quarter 3 after the k
            # projection.
            vctr = [0]

            def vproj(c):
                pv = pacc[:, vctr[0] % 4, :NF + 1]
                vctr[0] += 1
                sl = slice(c * PCH, (c + 1) * PCH)
                nc.tensor.matmul(pv, lhsT=s_rgb_lo[:, sl], rhs=s_wv0,
                                 start=True, stop=False)
                nc.tensor.matmul(pv, lhsT=s_rgb_hi[:, sl], rhs=s_wv1,
                                 start=False, stop=True)
                pcopy(out=s_vT[:, c, :], in_=pv)

            for c in range(3 * QUART):
                vproj(c)

            # k: for each column tile, accumulate 4 masked-weight matmuls
            # (quarter i lands in partition block i) then one full copy.
            for m, (k0, kw) in enumerate(zip(range(0, QSH, 512), JTILES)):
                pk = st[:, m % 4, :kw]
                for i in range(4):
                    sl = slice(i * QSH + k0, i * QSH + k0 + kw)
                    nc.tensor.matmul(pk, lhsT=s_wk0[:, i, :],
                                     rhs=s_rgb_lo[:, sl],
                                     start=(i == 0), stop=False)
                    nc.tensor.matmul(pk, lhsT=s_wk1[:, i, :],
                                     rhs=s_rgb_hi[:, sl],
                                     start=False, stop=(i == 3))
                pcopy(out=s_k4[:, k0:k0 + kw], in_=pk)

            for c in range(3 * QUART, NCH):
                vproj(c)

            # ---- attention ----
            j0 = 0
            for jt, JW in enumerate(JTILES):
                nslab = JW // 128

                def accum(e_t, g, nslab=nslab):
                    for i in range(4):
                        ch = QUART * i + g
                        for s in range(nslab):
                            nc.tensor.matmul(
                                pacc[:, s, :NF + 1],
                                lhsT=e_t[:, i, s * 128:(s + 1) * 128].bitcast(bf16),
                                rhs=s_vT[:, ch, :],
                                start=(g == 0 and i == 0),
                                stop=(g == QUART - 1 and i == 3),
                            )

                # software pipeline: scores(g) on 4 concurrent 32-row PE
                # tiles, exp(g) on VectorE, accum(g-1) on the full PE array
                e_prev = None
                for g in range(QUART):
                    for i in range(4):
                        nc.tensor.matmul(
                            st[:, i, :JW],
                            lhsT=s_k4[32 * i:32 * (i + 1), g * 128:(g + 1) * 128],
                            rhs=s_q4[32 * i:32 * (i + 1), j0:j0 + JW],
                            start=True, stop=True,
                            tile_position=(32 * i, 0),
                        )
                    e_t = work.tile([128, 4, 512], i16, tag="e",
                                    name=f"e_{jt}_{g}")
                    nc.vector.tensor_scalar(
                        out=e_t[:, :, :JW], in0=st[:, :, :JW],
                        scalar1=EXPA, scalar2=EXPB,
                        op0=mybir.AluOpType.mult, op1=mybir.AluOpType.add,
                    )
                    if e_prev is not None:
                        accum(e_prev, g - 1)
                    e_prev = e_t
                accum(e_prev, QUART - 1)

                # normalize: out = acc[:, :192] / acc[:, 192]; reciprocal on
                # VectorE (mandated), scale+copy on ScalarE (off the PE/DVE
                # critical path), DMA out.
                for s in range(nslab):
                    rec = work.tile([128, 1], f32, tag="rec",
                                    name=f"rec_{jt}_{s}", bufs=4)
                    nc.vector.reciprocal(rec, pacc[:, s, NF:NF + 1])
                    o_sb = work.tile([128, NF], f32, tag="osb",
                                     name=f"o_{jt}_{s}", bufs=4)
                    nc.scalar.mul(o_sb, pacc[:, s, :NF], rec)
                    r0 = j0 + s * 128
                    nc.sync.dma_start(out=d_out[r0:r0 + 128, :], in_=o_sb)
                j0 += JW

    nc.compile()
    return nc


def kernel(r, g, b, Wq, bq, Wk, bk, Wv, bv):
    global _last_results
    from concourse.bass_utils import run_bass_kernel_spmd

    r = np.asarray(r, np.float32)
    g = np.asarray(g, np.float32)
    b = np.asarray(b, np.float32)
    Wq = np.asarray(Wq, np.float32)
    bq = np.asarray(bq, np.float32)
    Wk = np.asarray(Wk, np.float32)
    bk = np.asarray(bk, np.float32)
    Wv = np.asarray(Wv, np.float32)
    bv = np.asarray(bv, np.float32)

    rgb = np.concatenate([r, g, b], axis=1).reshape(B, NF, N)  # fp32

    def bf(a):
        return np.ascontiguousarray(a).astype(BF)

    WqT = Wq.T  # [192, 24]
    WkT = Wk.T
    WvT = Wv.T  # [192, 192]

    # q weights: WqT replicated into all four 32-column blocks (cols
    # 32b..32b+23), bias in row 64 of the hi slab.
    wq0 = np.zeros((128, 128), np.float32)
    wq1 = np.zeros((KHI, 128), np.float32)
    for blk in range(4):
        wq0[:, 32 * blk:32 * blk + RD] = WqT[:128]
        wq1[:64, 32 * blk:32 * blk + RD] = WqT[128:]
        wq1[64, 32 * blk:32 * blk + RD] = bq
    # k weights: block-masked so quarter i lands in partition block i.
    wk0 = np.zeros((128, 4, 128), np.float32)
    wk1 = np.zeros((KHI, 4, 128), np.float32)
    for blk in range(4):
        wk0[:, blk, 32 * blk:32 * blk + RD] = WkT[:128]
        wk1[:64, blk, 32 * blk:32 * blk + RD] = WkT[128:]
        wk1[64, blk, 32 * blk:32 * blk + RD] = bk
    # v weights + ones column for the softmax denominator.
    wv0 = np.concatenate([WvT[:128], np.zeros((128, 1), np.float32)], axis=1)
    wv1 = np.concatenate(
        [np.concatenate([WvT[128:], np.zeros((64, 1), np.float32)], axis=1),
         np.concatenate([bv, np.ones(1, np.float32)])[None, :]], axis=0)

    wq0, wq1, wk0, wk1, wv0, wv1 = map(bf, (wq0, wq1, wk0, wk1, wv0, wv1))

    ones_n = np.ones((1, N), np.float32)
    in_maps = []
    for core in range(NCORES):
        bi = core // SHARDS_PER_BATCH
        j0 = (core % SHARDS_PER_BATCH) * SHARD
        rgb_b = rgb[bi]
        rgb_hi = np.concatenate([rgb_b[128:], ones_n], axis=0)
        in_maps.append({
            "rgb_lo": bf(rgb_b[:128]),
            "rgb_hi": bf(rgb_hi),
            "qrgb_lo": bf(rgb_b[:128, j0:j0 + SHARD]),
            "qrgb_hi": bf(rgb_hi[:, j0:j0 + SHARD]),
            "wq0": wq0, "wq1": wq1,
            "wk0": wk0, "wk1": wk1,
            "wv0": wv0, "wv1": wv1,
        })

    nc = _build_program()
    res = run_bass_kernel_spmd(nc, in_maps, list(range(NCORES)))
    _last_results = res

    att = np.empty((B, N, NF), np.float32)
    for core in range(NCORES):
        bi = core // SHARDS_PER_BATCH
        j0 = (core % SHARDS_PER_BATCH) * SHARD
        att[bi, j0:j0 + SHARD, :] = res.results[core]["out"]

    out = rgb + att.transpose(0, 2, 1)          # fp32 residual, exact
    out = out.reshape(B, NF, HH, WW)
    return (out[:, :C], out[:, C:2 * C], out[:, 2 * C:])


# revision 10
# speedup vs baseline: 1.1152x; 1.1152x over previous
"""Cross-channel attention kernel for Trainium2 (8 NeuronCores).

Problem (hardcoded shapes): B=2, C=64 per color -> NF=192 channels,
H=W=96 -> N=9216 spatial positions, RD=24 query/key dim.

    rgb  = concat(r,g,b)            # [B, 192, 9216]
    q    = Wq @ rgb + bq            # [B, 24, 9216]
    k    = Wk @ rgb + bk            # [B, 24, 9216]
    v    = Wv @ rgb + bv            # [B, 192, 9216]
    attn = softmax_j(q^T k)         # [B, 9216, 9216] row-softmax over keys
    out  = rgb + v @ attn^T         # residual added on host in fp32

Sharding: data-parallel over B (2) x sequence-parallel over query rows
(4 shards of 2304) = 8 cores.  Each core computes k and v redundantly
(they're tiny) and produces out[j, :] for its 2304 query rows.

Device-side layout ("keys on partitions"):
  scoresT[n, j] = sum_r k[r, n] q[r, j]     (K=24 matmul)
  e = exp(scoresT)                          (split across engines, below)
  acc[j, c] += e[n, j]^T vT[n, c_aug]       (matmul over key chunks of 128)
vT carries an all-ones column so acc[:, 192] accumulates the softmax
denominator; numerator and denominator ship to the host, which divides in
fp32 (frees ScalarE/VectorE at the j-tile boundaries).  No max-subtraction:
logits are O(1) by construction (weights scaled 0.02), exp cannot overflow.

Performance structure (things that made the naive version slow):
  * The scores matmul has K=24 -- a full 128x128 matmul wastes 4/5 of the
    PE array.  The PE is addressed in 32-row tiles (tile_position=(32i,0)):
    keys live in 4 quarter-blocks of 32 partitions (k4), q is replicated
    into all four 32-partition blocks (q4), and the 4 score matmuls of a
    group run CONCURRENTLY on the four row-groups of the array, writing 4
    different PSUM banks (~3x measured on this idiom).
  * exp of each group is split across BOTH activation-capable engines in
    parallel on disjoint PSUM banks: ScalarE does true Exp on chunks 0-1,
    VectorE does Schraudolph fast-exp (int16 affine -> bf16 bit pattern,
    +-3% per element, cancels in the softmax ratio) on chunks 2-3.  Each
    engine's half-group (1024 elem/partition) takes ~1.0/1.1us against
    ~1.5us of PE work per group, so the single-buffered score banks never
    stall the PE and the HAM clock stays at 2.4 GHz.
  * Projection weights are replicated/masked host-side into 32-column
    blocks so every PSUM->SBUF copy is full 128-partition width; the
    hi-slab of rgb carries an all-ones row 64 (host-side) for the biases;
    v-chunk copies are batched 4 chunks per instruction; copies alternate
    Vector/Scalar.  No device memsets, no 32-partition copies.
  * PSUM: scores 4 banks + accumulators 4 banks (one 193-wide region per
    bank) = exactly 8 banks.
"""

import numpy as np
import ml_dtypes

BF = ml_dtypes.bfloat16

# Shapes (hardcoded per problem spec)
B = 2
C = 64
HH = 96
WW = 96
N = HH * WW            # 9216 keys
NF = 3 * C             # 192 channels
RD = 24                # q/k dim
NCORES = 8
SHARDS_PER_BATCH = 4
SHARD = N // SHARDS_PER_BATCH   # 2304 query rows per core

JTILES = [512, 512, 512, 512, 256]   # query-tile widths (sum = SHARD)
PCH = 128              # key chunk (partition dim)
NCH = N // PCH         # 72 key chunks
QUART = NCH // 4       # 18 chunks per key quarter
KHI = 65               # hi K-slab: channels 128..191 + ones row
QSH = N // 4           # 2304 keys per quarter
NFA = NF + 1           # 193: channels + denominator column

_last_results = None   # BassKernelResults of the most recent run (for test.py)


def _build_program():
    import concourse.tile as tile
    from concourse import bacc, mybir

    f32 = mybir.dt.float32
    bf16 = mybir.dt.bfloat16
    i16 = mybir.dt.int16
    Exp = mybir.ActivationFunctionType.Exp
    # Schraudolph fast-exp in bf16 bit space:
    #   exp(x) ~= bitcast_bf16(int16(A*x + B)),  A = 2^7/ln2, B = 127*2^7 - c
    EXPA = float(128.0 / np.log(2.0))
    EXPB = float(127 * 128) - 5.59

    nc = bacc.Bacc()

    # rgb hi slabs carry an all-ones row 64 (built host-side) for the biases
    d_rgb_lo = nc.dram_tensor("rgb_lo", [128, N], bf16, kind="ExternalInput")
    d_rgb_hi = nc.dram_tensor("rgb_hi", [KHI, N], bf16, kind="ExternalInput")
    d_qrgb_lo = nc.dram_tensor("qrgb_lo", [128, SHARD], bf16, kind="ExternalInput")
    d_qrgb_hi = nc.dram_tensor("qrgb_hi", [KHI, SHARD], bf16, kind="ExternalInput")
    # q/k weight slabs: 4 replicated (q) or masked (k) 32-column blocks;
    # hi slabs carry the bias in row 64 (multiplied by the rgb ones row)
    d_wq0 = nc.dram_tensor("wq0", [128, 128], bf16, kind="ExternalInput")
    d_wq1 = nc.dram_tensor("wq1", [KHI, 128], bf16, kind="ExternalInput")
    d_wk0 = nc.dram_tensor("wk0", [128, 4, 128], bf16, kind="ExternalInput")
    d_wk1 = nc.dram_tensor("wk1", [KHI, 4, 128], bf16, kind="ExternalInput")
    d_wv0 = nc.dram_tensor("wv0", [128, NFA], bf16, kind="ExternalInput")
    d_wv1 = nc.dram_tensor("wv1", [KHI, NFA], bf16, kind="ExternalInput")
    d_out = nc.dram_tensor("out", [SHARD, NFA], f32, kind="ExternalOutput")

    with tile.TileContext(nc) as tc:
        with (
            tc.tile_pool(name="const", bufs=1) as const,
            tc.tile_pool(name="work", bufs=3) as work,
            tc.tile_pool(name="pp", bufs=1, space="PSUM") as pp,
        ):
            # ---- SBUF tiles ----
            s_rgb_lo = const.tile([128, N], bf16)
            s_rgb_hi = const.tile([KHI, N], bf16)
            s_qrgb_lo = const.tile([128, SHARD], bf16)
            s_qrgb_hi = const.tile([KHI, SHARD], bf16)
            s_wq0 = const.tile([128, 128], bf16)
            s_wq1 = const.tile([KHI, 128], bf16)
            s_wk0 = const.tile([128, 4, 128], bf16)
            s_wk1 = const.tile([KHI, 4, 128], bf16)
            s_wv0 = const.tile([128, NFA], bf16)
            s_wv1 = const.tile([KHI, NFA], bf16)
            # k4: partition block i = key quarter i (rows: 24 k dims + 8 zero)
            # q4: q replicated in all 4 partition blocks
            s_k4 = const.tile([128, QSH], bf16)
            s_q4 = const.tile([128, SHARD], bf16)
            s_vT = const.tile([128, NCH, NFA], bf16)

            # PSUM: tag "s" = 4 banks (scores; proj targets), tag "acc" =
            # 4 banks (attention accumulators, one 193-wide region per bank;
            # proj targets before that).  Both tiles live the whole kernel.
            st = pp.tile([128, 4, 512], f32, tag="s", bufs=1, name="st")
            pacc = pp.tile([128, 4, 512], f32, tag="acc", bufs=1, name="pacc")

            # ---- input DMA ----
            nc.sync.dma_start(out=s_wq0[:], in_=d_wq0[:])
            nc.sync.dma_start(out=s_wq1[:], in_=d_wq1[:])
            nc.sync.dma_start(out=s_wk0[:], in_=d_wk0[:])
            nc.sync.dma_start(out=s_wk1[:], in_=d_wk1[:])
            nc.sync.dma_start(out=s_wv0[:], in_=d_wv0[:])
            nc.sync.dma_start(out=s_wv1[:], in_=d_wv1[:])
            for i in range(2):
                sl = slice(i * (SHARD // 2), (i + 1) * (SHARD // 2))
                nc.sync.dma_start(out=s_qrgb_lo[:, sl], in_=d_qrgb_lo[:, sl])
                nc.sync.dma_start(out=s_qrgb_hi[:, sl], in_=d_qrgb_hi[:, sl])
            for i in range(4):
                sl = slice(i * QSH, (i + 1) * QSH)
                nc.sync.dma_start(out=s_rgb_lo[:, sl], in_=d_rgb_lo[:, sl])
                nc.sync.dma_start(out=s_rgb_hi[:, sl], in_=d_rgb_hi[:, sl])

            # PE warmup: the HAM clock gate keeps the PE at 1.2 GHz until it
            # sees a ~3.4us busy window.  Burn zero matmuls under the input
            # DMA head so the projections run at 2.4 GHz.
            wz = const.tile([128, 512], bf16)
            nc.vector.memset(wz, 0.0)
            for w in range(10):
                nc.tensor.matmul(st[:, w % 4, :], lhsT=wz[:, :128], rhs=wz,
                                 start=True, stop=True)

            # rotate PSUM->SBUF copies between Vector and Scalar
            cctr = [0]

            def pcopy(out, in_):
                if cctr[0] % 2 == 0:
                    nc.vector.tensor_copy(out=out, in_=in_)
                else:
                    nc.scalar.copy(out=out, in_=in_)
                cctr[0] += 1

            # ---- projections ----
            # q: replicated weights -> q in all 4 partition blocks; one
            # full-width copy per tile.
            for m, (q0, qw) in enumerate(zip(range(0, SHARD, 512), JTILES)):
                sl = slice(q0, q0 + qw)
                pq = st[:, m % 4, :qw]
                nc.tensor.matmul(pq, lhsT=s_wq0, rhs=s_qrgb_lo[:, sl],
                                 start=True, stop=False)
                nc.tensor.matmul(pq, lhsT=s_wq1, rhs=s_qrgb_hi[:, sl],
                                 start=False, stop=True)
                pcopy(out=s_q4[:, sl], in_=pq)

            # v: vT[key, c] per key chunk + ones column.  Batches of up to
            # 4 chunks fill the 4 banks of one PSUM tile (alternating
            # pacc / st for double-buffering), then ONE batched copy.
            # Quarters 0-2 first (overlapping the rgb DMA), quarter 3 after
            # the k projection.
            vbctr = [0]

            def vproj_batch(c0, nch_b):
                tgt = [pacc, st][vbctr[0] % 2]
                vbctr[0] += 1
                for kk in range(nch_b):
                    c = c0 + kk
                    pv = tgt[:, kk, :NFA]
                    sl = slice(c * PCH, (c + 1) * PCH)
                    nc.tensor.matmul(pv, lhsT=s_rgb_lo[:, sl], rhs=s_wv0,
                                     start=True, stop=False)
                    nc.tensor.matmul(pv, lhsT=s_rgb_hi[:, sl], rhs=s_wv1,
                                     start=False, stop=True)
                pcopy(out=s_vT[:, c0:c0 + nch_b, :], in_=tgt[:, :nch_b, :NFA])

            def vproj_quarter(qi):
                base = QUART * qi
                for b0 in range(0, QUART, 4):
                    vproj_batch(base + b0, min(4, QUART - b0))

            for qi in range(3):
                vproj_quarter(qi)

            # k: for each column tile, accumulate 4 masked-weight matmuls
            # (quarter i lands in partition block i) then one full copy.
            for m, (k0, kw) in enumerate(zip(range(0, QSH, 512), JTILES)):
                pk = pacc[:, m % 4, :kw]
                for i in range(4):
                    sl = slice(i * QSH + k0, i * QSH + k0 + kw)
                    nc.tensor.matmul(pk, lhsT=s_wk0[:, i, :],
                                     rhs=s_rgb_lo[:, sl],
                                     start=(i == 0), stop=False)
                    nc.tensor.matmul(pk, lhsT=s_wk1[:, i, :],
                                     rhs=s_rgb_hi[:, sl],
                                     start=False, stop=(i == 3))
                pcopy(out=s_k4[:, k0:k0 + kw], in_=pk)

            vproj_quarter(3)

            # ---- attention ----
            j0 = 0
            for jt, JW in enumerate(JTILES):
                nslab = JW // 128

                def accum(e_t, g, nslab=nslab):
                    for i in range(4):
                        ch = QUART * i + g
                        for s in range(nslab):
                            nc.tensor.matmul(
                                pacc[:, s, :NFA],
                                lhsT=e_t[:, i, s * 128:(s + 1) * 128],
                                rhs=s_vT[:, ch, :],
                                start=(g == 0 and i == 0),
                                stop=(g == QUART - 1 and i == 3),
                            )

                # per group: 4 concurrent 32-row score matmuls -> ScalarE
                # true-exp on banks 0-1 || VectorE Schraudolph on banks 2-3
                # -> previous group's accumulation matmuls on the full array
                e_prev = None
                for g in range(QUART):
                    for i in range(4):
                        nc.tensor.matmul(
                            st[:, i, :JW],
                            lhsT=s_k4[32 * i:32 * (i + 1), g * 128:(g + 1) * 128],
                            rhs=s_q4[32 * i:32 * (i + 1), j0:j0 + JW],
                            start=True, stop=True,
                            tile_position=(32 * i, 0),
                        )
                    e_t = work.tile([128, 4, 512], bf16, tag="e",
                                    name=f"e_{jt}_{g}")
                    nc.scalar.activation(out=e_t[:, 0:2, :JW],
                                         in_=st[:, 0:2, :JW], func=Exp)
                    nc.vector.tensor_scalar(
                        out=e_t[:, 2:4, :JW].bitcast(i16),
                        in0=st[:, 2:4, :JW],
                        scalar1=EXPA, scalar2=EXPB,
                        op0=mybir.AluOpType.mult, op1=mybir.AluOpType.add,
                    )
                    if e_prev is not None:
                        accum(e_prev, g - 1)
                    e_prev = e_t
                accum(e_prev, QUART - 1)

                # ship numerator + denominator; host divides in fp32
                for s in range(nslab):
                    o_sb = work.tile([128, NFA], f32, tag="osb",
                                     name=f"o_{jt}_{s}", bufs=4)
                    if s % 2 == 0:
                        nc.vector.tensor_copy(out=o_sb, in_=pacc[:, s, :NFA])
                    else:
                        nc.scalar.copy(out=o_sb, in_=pacc[:, s, :NFA])
                    r0 = j0 + s * 128
                    nc.sync.dma_start(out=d_out[r0:r0 + 128, :], in_=o_sb)
                j0 += JW

    nc.compile()
    return nc


def kernel(r, g, b, Wq, bq, Wk, bk, Wv, bv):
    global _last_results
    from concourse.bass_utils import run_bass_kernel_spmd

    r = np.asarray(r, np.float32)
    g = np.asarray(g, np.float32)
    b = np.asarray(b, np.float32)
    Wq = np.asarray(Wq, np.float32)
    bq = np.asarray(bq, np.float32)
    Wk = np.asarray(Wk, np.float32)
    bk = np.asarray(bk, np.float32)
    Wv = np.asarray(Wv, np.float32)
    bv = np.asarray(bv, np.float32)

    rgb = np.concatenate([r, g, b], axis=1).reshape(B, NF, N)  # fp32

    def bf(a):
        return np.ascontiguousarray(a).astype(BF)

    WqT = Wq.T  # [192, 24]
    WkT = Wk.T
    WvT = Wv.T  # [192, 192]

    # q weights: WqT replicated into all four 32-column blocks (cols
    # 32b..32b+23), bias in row 64 of the hi slab.
    wq0 = np.zeros((128, 128), np.float32)
    wq1 = np.zeros((KHI, 128), np.float32)
    for blk in range(4):
        wq0[:, 32 * blk:32 * blk + RD] = WqT[:128]
        wq1[:64, 32 * blk:32 * blk + RD] = WqT[128:]
        wq1[64, 32 * blk:32 * blk + RD] = bq
    # k weights: block-masked so quarter i lands in partition block i.
    wk0 = np.zeros((128, 4, 128), np.float32)
    wk1 = np.zeros((KHI, 4, 128), np.float32)
    for blk in range(4):
        wk0[:, blk, 32 * blk:32 * blk + RD] = WkT[:128]
        wk1[:64, blk, 32 * blk:32 * blk + RD] = WkT[128:]
        wk1[64, blk, 32 * blk:32 * blk + RD] = bk
    # v weights + ones column for the softmax denominator.
    wv0 = np.concatenate([WvT[:128], np.zeros((128, 1), np.float32)], axis=1)
    wv1 = np.concatenate(
        [np.concatenate([WvT[128:], np.zeros((64, 1), np.float32)], axis=1),
         np.concatenate([bv, np.ones(1, np.float32)])[None, :]], axis=0)

    wq0, wq1, wk0, wk1, wv0, wv1 = map(bf, (wq0, wq1, wk0, wk1, wv0, wv1))

    ones_n = np.ones((1, N), np.float32)
    in_maps = []
    for core in range(NCORES):
        bi = core // SHARDS_PER_BATCH
        j0 = (core % SHARDS_PER_BATCH) * SHARD
        rgb_b = rgb[bi]
        rgb_hi = np.concatenate([rgb_b[128:], ones_n], axis=0)
        in_maps.append({
            "rgb_lo": bf(rgb_b[:128]),
            "rgb_hi": bf(rgb_hi),
            "qrgb_lo": bf(rgb_b[:128, j0:j0 + SHARD]),
            "qrgb_hi": bf(rgb_hi[:, j0:j0 + SHARD]),
            "wq0": wq0, "wq1": wq1,
            "wk0": wk0, "wk1": wk1,
            "wv0": wv0, "wv1": wv1,
        })

    nc = _build_program()
    res = run_bass_kernel_spmd(nc, in_maps, list(range(NCORES)))
    _last_results = res

    att = np.empty((B, N, NF), np.float32)
    for core in range(NCORES):
        bi = core // SHARDS_PER_BATCH
        j0 = (core % SHARDS_PER_BATCH) * SHARD
        o = res.results[core]["out"]          # [SHARD, 193] num | denom
        att[bi, j0:j0 + SHARD, :] = o[:, :NF] / o[:, NF:NF + 1]

    out = rgb + att.transpose(0, 2, 1)          # fp32 residual, exact
    out = out.reshape(B, NF, HH, WW)
    return (out[:, :C], out[:, C:2 * C], out[:, 2 * C:])
